# revision 7
# baseline (speedup 1.0000x reference)
"""JPEG encode/decode kernel for Trainium2 (Bass/Tile), 8-core data parallel.

Self-contained: hardcodes shapes [8,3,1024,1024] f32 + [8] f32 and all constants.

Dataflow per core (1 image), per [128,512] group (8 row-strips x 2 col-halves):
  A (PE, swap):  A = (Dv@Y)^T and (Ev@Cb|Cr)^T fused with RGB->YCC, 12 MM N=256
  B (PE, const): C^T = Dv@A_y (N=512), chroma C^T = Ev@A_ch (N=512)
  quant (DVE):   u = C^T - 1024*DCmask; z = custom_diffround(u*qinv); C'' = z*qt
  C (PE, swap):  C1 = C''@Dv (Y), C1 = C''@F^T (chroma: IDCT+upsample horizontal)
  D (PE, const): psum_{R,G,B} = Dv^T@C1y + W_ch@C1ch (vertical IDCT+upsample+color)
  finals:        ACT relu(psum+128) -> GPSIMD/DVE min 255 -> DMA out
"""
import numpy as np

H = W = 1024
NCORES = 8
MAGIC = float(np.float32(1.5 * 2 ** 23))

# ---------------------------------------------------------------- constants
QT_Y8 = np.array([
    [16, 11, 10, 16, 24, 40, 51, 61],
    [12, 12, 14, 19, 26, 58, 60, 55],
    [14, 13, 16, 24, 40, 57, 69, 56],
    [14, 17, 22, 29, 51, 87, 80, 62],
    [18, 22, 37, 56, 68, 109, 103, 77],
    [24, 35, 55, 64, 81, 104, 113, 92],
    [49, 64, 78, 87, 103, 121, 120, 101],
    [72, 92, 95, 98, 112, 100, 103, 99]], dtype=np.float32)
QT_C8 = np.array([
    [17, 18, 24, 47, 99, 99, 99, 99],
    [18, 21, 26, 66, 99, 99, 99, 99],
    [24, 26, 56, 99, 99, 99, 99, 99],
    [47, 66, 99, 99, 99, 99, 99, 99],
    [99, 99, 99, 99, 99, 99, 99, 99],
    [99, 99, 99, 99, 99, 99, 99, 99],
    [99, 99, 99, 99, 99, 99, 99, 99],
    [99, 99, 99, 99, 99, 99, 99, 99]], dtype=np.float32)


def _host_constants():
    k = np.arange(8)[:, None]
    n = np.arange(8)[None, :]
    D = np.sqrt(2.0 / 8.0) * np.cos((2 * n + 1) * k * np.pi / 16.0)
    D[0, :] *= 1.0 / np.sqrt(2.0)
    D32 = D.astype(np.float32)
    D64 = D32.astype(np.float64)

    avg2 = np.zeros((8, 16))
    up2 = np.zeros((16, 8))
    for i in range(8):
        avg2[i, 2 * i] = avg2[i, 2 * i + 1] = 0.5
        up2[2 * i, i] = up2[2 * i + 1, i] = 1.0
    E = (D64 @ avg2).astype(np.float32)          # [8,16]
    F = (up2 @ D64.T).astype(np.float32)         # [16,8]

    def blockdiag(M, cnt):
        r, c = M.shape
        out = np.zeros((r * cnt, c * cnt), M.dtype)
        for i in range(cnt):
            out[i * r:(i + 1) * r, i * c:(i + 1) * c] = M
        return out

    Dv = blockdiag(D32, 16)      # [128,128]
    Ev = blockdiag(E, 8)         # [64,128]
    Fv = blockdiag(F, 8)         # [128,64]

    WY = np.array([0.299, 0.587, 0.114], np.float64)
    WCB = np.array([-0.168736, -0.331264, 0.5], np.float64)
    WCR = np.array([0.5, -0.418688, -0.081312], np.float64)

    DvT64 = Dv.T.astype(np.float64)
    EvT64 = Ev.T.astype(np.float64)
    FvT64 = Fv.T.astype(np.float64)

    # stage A rhs, per input channel: [128, 256]
    cA = np.stack([np.concatenate([(WY[c] * DvT64).astype(np.float32),
                                   (WCB[c] * EvT64).astype(np.float32),
                                   (WCR[c] * EvT64).astype(np.float32)], axis=1)
                   for c in range(3)], axis=0)          # [3,128,256]
    cA = cA.reshape(384, 256)
    # stage B lhsT: [128, 192] = [Dv.T | Ev.T]
    cB = np.concatenate([Dv.T, Ev.T], axis=1).astype(np.float32)
    # stage C rhs: [128, 256] = [Dv | Fv.T (rows 0:64)]
    cC = np.zeros((128, 256), np.float32)
    cC[:, 0:128] = Dv
    cC[0:64, 128:256] = Fv.T
    # stage D lhsT: [128, 512] = [Dv | LD_R | LD_G | LD_B]
    Z64 = np.zeros((64, 128), np.float64)
    LD_R = np.concatenate([Z64, 1.402 * FvT64], axis=0)
    LD_G = np.concatenate([-0.344136 * FvT64, -0.714136 * FvT64], axis=0)
    LD_B = np.concatenate([1.772 * FvT64, Z64], axis=0)
    cD = np.concatenate([Dv.astype(np.float64), LD_R, LD_G, LD_B],
                        axis=1).astype(np.float32)      # [128,512]
    return cA, cB, cC, cD


def _qt_tables(q):
    q = np.float32(q)
    if q < np.float32(50.0):
        s = np.float32(5000.0) / q
    else:
        s = np.float32(200.0) - np.float32(2.0) * q
    qy = np.clip(np.floor((QT_Y8 * s + np.float32(50.0)) / np.float32(100.0)),
                 np.float32(1.0), np.float32(255.0)).astype(np.float32)
    qc = np.clip(np.floor((QT_C8 * s + np.float32(50.0)) / np.float32(100.0)),
                 np.float32(1.0), np.float32(255.0)).astype(np.float32)
    return qy, qc


def _quant_consts(qy, qc):
    """cQ [128,40]: qinv_y | qmul_y | amask | qinv_c | qmul_c (8 cols each).

    Quant operates on C^T tiles: [partition p = col-freq l, free k], divisor
    qt[k%8, p%8]."""
    p = np.arange(128) % 8
    i8 = np.arange(8)[None, :]
    qinv_y = (1.0 / qy[i8, p[:, None]].astype(np.float64)).astype(np.float32)
    qmul_y = qy[i8, p[:, None]].astype(np.float32)
    qinv_c = (1.0 / qc[i8, p[:, None]].astype(np.float64)).astype(np.float32)
    qmul_c = qc[i8, p[:, None]].astype(np.float32)
    amask = np.zeros((128, 8), np.float32)
    amask[p == 0, 0] = 1024.0
    bias = np.full((128, 8), 128.0, np.float32)
    return np.concatenate([qinv_y, qmul_y, amask, qinv_c, qmul_c, bias], axis=1)


# ---------------------------------------------------------------- custom DVE op
def _register_diffround_op():
    import concourse.dve_ops as dve_ops
    from concourse.dve_spec import Spec, Src0, Src1, C0, sq, lower
    from concourse.dve_uop import DveOpSpec

    name = "JPEG_DIFF_ROUND"
    for op in dve_ops.OPS:
        if op.name == name:
            return op

    t = Src0 * Src1
    r = (t + C0) - C0
    f = t - r
    body = t - (f - sq(f) * f)

    def _ref(in0, in1, s0):
        t = (in0 * in1).astype(np.float32)
        r = np.round(t)
        f = t - r
        return t - (f - f * f * f)

    spec = Spec(body=body, reference=_ref)
    row = 1 + len(dve_ops.OPS)
    dve_ops._SUB_OPCODE_FOR_NAME[name] = row
    shas = {}
    for ver in ("v3", "v4"):
        s = DveOpSpec(name=name, opcode=row, uops=lower(spec, ver=ver), rd1_en=True)
        shas[ver] = s.sha(ver)
    op = dve_ops.DveOp(name, spec, subdim=False, uops_sha=shas)
    dve_ops.OPS.append(op)
    dve_ops.CUSTOM_DVE_SPECS[name] = spec
    return op


# ---------------------------------------------------------------- bass kernel
_CACHE = {}


def _build_kernel():
    if "nc" in _CACHE:
        return _CACHE["nc"]
    import concourse.bass as bass  # noqa: F401
    import concourse.mybir as mybir
    import concourse.tile as tile
    from concourse import bacc

    op = _register_diffround_op()
    f32 = mybir.dt.float32
    Alu = mybir.AluOpType
    Act = mybir.ActivationFunctionType

    nc = bacc.Bacc(None, target_bir_lowering=False)
    rgb = nc.dram_tensor("rgb", [3, H, W], f32, kind="ExternalInput")
    cA_d = nc.dram_tensor("cA", [384, 256], f32, kind="ExternalInput")
    cB_d = nc.dram_tensor("cB", [128, 192], f32, kind="ExternalInput")
    cC_d = nc.dram_tensor("cC", [128, 256], f32, kind="ExternalInput")
    cD_d = nc.dram_tensor("cD", [128, 512], f32, kind="ExternalInput")
    cQ_d = nc.dram_tensor("cQ", [128, 48], f32, kind="ExternalInput")
    out = nc.dram_tensor("out", [3, H, W], f32, kind="ExternalOutput")

    def b8(ap2d, parts):
        """[P,8] AP -> [P,64,8] broadcast view along free dim."""
        return ap2d.unsqueeze(1).broadcast_to((parts, 64, 8))

    def v3(ap2d, parts):
        """[P,512] AP -> [P,64,8] view."""
        return ap2d.rearrange("p (a b) -> p a b", b=8)

    with tile.TileContext(nc) as tc:
        with (
            tc.tile_pool(name="consts", bufs=1) as cpool,
            tc.tile_pool(name="xin", bufs=6) as xpool,
            tc.tile_pool(name="sA", bufs=2) as sapool,
            tc.tile_pool(name="qY", bufs=2) as qypool,
            tc.tile_pool(name="qC", bufs=2) as qcpool,
            tc.tile_pool(name="c1", bufs=2) as c1pool,
            tc.tile_pool(name="outs", bufs=6) as opool,
            tc.tile_pool(name="pA", bufs=1, space="PSUM") as pApool,
            tc.tile_pool(name="pB", bufs=1, space="PSUM") as pBpool,
            tc.tile_pool(name="pC", bufs=1, space="PSUM") as pCpool,
            tc.tile_pool(name="pO", bufs=1, space="PSUM") as pOpool,
        ):
            # 384-partition DRAM -> 3 SBUF loads of 128
            cA_t = [cpool.tile([128, 256], f32, tag=f"cAs{c}", name=f"cAs{c}") for c in range(3)]
            for c in range(3):
                nc.sync.dma_start(cA_t[c][:, :], cA_d[128 * c:128 * (c + 1), :])
            cB_t = cpool.tile([128, 192], f32, tag="cB")
            nc.sync.dma_start(cB_t[:, :], cB_d[:, :])
            cC_t = cpool.tile([128, 256], f32, tag="cC")
            nc.sync.dma_start(cC_t[:, :], cC_d[:, :])
            cD_t = cpool.tile([128, 512], f32, tag="cD")
            nc.sync.dma_start(cD_t[:, :], cD_d[:, :])
            cQ_t = cpool.tile([128, 48], f32, tag="cQ")
            nc.sync.dma_start(cQ_t[:, :], cQ_d[:, :])

            qinv_y = cQ_t[:, 0:8]
            qmul_y = cQ_t[:, 8:16]
            amask = cQ_t[:, 16:24]
            qinv_c = cQ_t[0:64, 24:32]
            qmul_c = cQ_t[0:64, 32:40]
            bias128 = cQ_t[:, 40:41]

            for s in range(8):
                r0 = 128 * s
                for hh in range(2):
                    c0 = 512 * hh
                    # ---- DMA in
                    X = []
                    for c in range(3):
                        xt = xpool.tile([128, 512], f32, tag=f"x{c}", name=f"x{c}")
                        nc.sync.dma_start(xt[:, :], rgb[c, r0:r0 + 128, c0:c0 + 512])
                        X.append(xt)
                    # ---- stage A: 2 psum tiles, each 2 regions
                    sA = sapool.tile([128, 1024], f32, tag="sA")
                    for half in range(2):
                        pA = pApool.tile([128, 512], f32, tag="pA")
                        for rr in range(2):
                            r = 2 * half + rr
                            for c in range(3):
                                nc.tensor.matmul(
                                    pA[:, 256 * rr:256 * rr + 256],
                                    X[c][:, 128 * r:128 * r + 128],
                                    cA_t[c][:, :],
                                    start=(c == 0), stop=(c == 2))
                        nc.scalar.activation(sA[:, 512 * half:512 * half + 512],
                                             pA[:, :], Act.Copy)
                    # ---- stage B
                    sA3 = sA[:, :].rearrange("p (r x) -> p r x", x=256)
                    pBy = pBpool.tile([128, 512], f32, tag="pBy")
                    nc.tensor.matmul(pBy[:, :], cB_t[:, 0:128], sA3[:, :, 0:128],
                                     start=True, stop=True)
                    pBc = pBpool.tile([64, 512], f32, tag="pBc")
                    nc.tensor.matmul(pBc[:, :], cB_t[:, 128:192], sA3[:, :, 128:256],
                                     start=True, stop=True)
                    # ---- quant Y: u = C - A; z = custom(u*qinv); C'' = z*qt
                    uY = qypool.tile([128, 512], f32, tag="uY")
                    nc.vector.scalar_tensor_tensor(
                        v3(uY[:, :], 128), v3(pBy[:, :], 128), 0.0,
                        b8(amask, 128), Alu.bypass, Alu.subtract)
                    zY = qypool.tile([128, 512], f32, tag="zY")
                    nc.vector._custom_dve(op, out=v3(zY[:, :], 128),
                                          in0=v3(uY[:, :], 128),
                                          in1=b8(qinv_y, 128), s0=MAGIC)
                    CqY = qypool.tile([128, 512], f32, tag="CqY")
                    nc.vector.scalar_tensor_tensor(
                        v3(CqY[:, :], 128), v3(zY[:, :], 128), 0.0,
                        b8(qmul_y, 128), Alu.bypass, Alu.mult)
                    # ---- quant chroma (no DC offset)
                    uC = qcpool.tile([64, 512], f32, tag="uC")
                    nc.vector.tensor_scalar(uC[:, :], pBc[:, :], 0.0, None, Alu.bypass)
                    zC = qcpool.tile([64, 512], f32, tag="zC")
                    nc.vector._custom_dve(op, out=v3(zC[:, :], 64),
                                          in0=v3(uC[:, :], 64),
                                          in1=b8(qinv_c, 64), s0=MAGIC)
                    CqC = qcpool.tile([64, 512], f32, tag="CqC")
                    nc.vector.scalar_tensor_tensor(
                        v3(CqC[:, :], 64), v3(zC[:, :], 64), 0.0,
                        b8(qmul_c, 64), Alu.bypass, Alu.mult)
                    # ---- stage C (per region)
                    pCy = pCpool.tile([128, 512], f32, tag="pCy")
                    pCc = pCpool.tile([128, 512], f32, tag="pCc")
                    for r in range(4):
                        sl = slice(128 * r, 128 * r + 128)
                        nc.tensor.matmul(pCy[:, sl], CqY[:, sl], cC_t[:, 0:128],
                                         start=True, stop=True)
                        nc.tensor.matmul(pCc[:, sl], CqC[:, sl], cC_t[0:64, 128:256],
                                         start=True, stop=True)
                    sC1y = c1pool.tile([128, 512], f32, tag="sC1y")
                    nc.vector.tensor_scalar(sC1y[:, :], pCy[:, :], 0.0, None, Alu.bypass)
                    sC1c = c1pool.tile([128, 512], f32, tag="sC1c")
                    nc.vector.tensor_scalar(sC1c[:, :], pCc[:, :], 0.0, None, Alu.bypass)
                    # ---- stage D: 3 channel psums
                    pOuts = []
                    for ch in range(3):
                        po = pOpool.tile([128, 512], f32, tag=f"pO{ch}", name=f"pO{ch}")
                        pOuts.append(po)
                    for ch in range(3):
                        nc.tensor.matmul(pOuts[ch][:, :], cD_t[:, 0:128], sC1y[:, :],
                                         start=True, stop=False)
                    for ch in range(3):
                        sl = slice(128 * (ch + 1), 128 * (ch + 2))
                        nc.tensor.matmul(pOuts[ch][:, :], cD_t[:, sl], sC1c[:, :],
                                         start=False, stop=True)
                    # ---- finals: relu(x+128) then min 255, DMA out
                    for ch in range(3):
                        so = opool.tile([128, 512], f32, tag=f"so{ch}", name=f"so{ch}")
                        nc.scalar.activation(so[:, :], pOuts[ch][:, :], Act.Relu,
                                             bias=bias128)
                        so2 = opool.tile([128, 512], f32, tag=f"sm{ch}", name=f"sm{ch}")
                        nc.gpsimd.tensor_scalar(so2[:, :], so[:, :], 255.0, None,
                                                Alu.min)
                        nc.sync.dma_start(out[ch, r0:r0 + 128, c0:c0 + 512],
                                          so2[:, :])
    nc.finalize()
    _CACHE["nc"] = nc
    return nc


def kernel(image_rgb: np.ndarray, compression_strength: np.ndarray) -> np.ndarray:
    from concourse.bass_utils import run_bass_kernel_spmd

    nc = _build_kernel()
    cA, cB, cC, cD = _CACHE.setdefault("consts", _host_constants())
    image_rgb = np.ascontiguousarray(np.asarray(image_rgb, dtype=np.float32))
    q = np.asarray(compression_strength, dtype=np.float32)

    in_maps = []
    for b in range(NCORES):
        qy, qc = _qt_tables(q[b])
        cQ = _quant_consts(qy, qc)
        in_maps.append({"rgb": image_rgb[b], "cA": cA, "cB": cB, "cC": cC,
                        "cD": cD, "cQ": cQ})
    res = run_bass_kernel_spmd(nc, in_maps, core_ids=list(range(NCORES)))
    return np.stack([res.results[b]["out"] for b in range(NCORES)], axis=0)
